# revision 1
# baseline (speedup 1.0000x reference)
"""2-layer GCN (PyG GCNConv x2, eval mode) on 8 TRN2 NeuronCores, SPMD.

Strategy (graph-partition data parallel, per the sharding hint):
  - Nodes are sharded contiguously across the 8 cores (dst ownership).
  - Per layer:  t = X @ W ; g = t * dinv  (D^-1/2 factored per-node),
    g is cast to bf16 and AllGathered so every core holds the full node table;
    each core then aggregates its owned destinations:
        q[v] = sum_{e: dst=v} g[src]   (self loops are host-added edges)
    and applies the tail (z = q*dinv + b, relu for layer 1).
  - Aggregation engine: edges are bucketed host-side by
    (dst-block of 128, src range of V/4) so the custom `dma_gather`
    (int16 indices, 256B bf16 rows) can pull message rows into SBUF
    partition-major tiles; the DVE builds a one-hot S = (iota == dst_local)
    per 128-edge tile and the PE accumulates S^T @ rows into PSUM per dst
    block (segment-sum as matmul).  Pad slots gather row 0 and carry
    dst_local -1 (zero one-hot column), so they contribute nothing.
  - Layer 2 output (C=2) is formed as two masked row-reductions on the DVE;
    b2 is added on the host.
"""

import numpy as np
import ml_dtypes

import concourse.bass as bass
import concourse.mybir as mybir
import concourse.tile as tile
import concourse.bacc as bacc
from concourse.bass_utils import run_bass_kernel_spmd

F32 = mybir.dt.float32
BF16 = mybir.dt.bfloat16
I16 = mybir.dt.int16

HID = 128
P = 128
NRANGE = 4
N_NODES = 100000
F_IN = 165
CORES = 8
NB = 100          # dst blocks per core (12800 padded nodes/core)
GB = 5            # dst blocks per gather group
NSH = 12500
NSH_PAD = NB * P
V_PAD = CORES * NSH_PAD
RANGE = V_PAD // NRANGE
NG = NB // GB


def _edge_structure(src, dst):
    csh = np.minimum(dst // NSH, CORES - 1)
    r_loc = dst - csh * NSH
    blk = r_loc // P
    dloc = r_loc % P
    csrc = np.minimum(src // NSH, CORES - 1)
    grow = csrc * NSH_PAD + (src - csrc * NSH)
    rng = grow // RANGE
    grp = blk // GB
    dl = blk % GB
    key = ((csh * NG + grp) * NRANGE + rng) * GB + dl
    nkeys = CORES * NG * NRANGE * GB
    counts = np.bincount(key, minlength=nkeys).reshape(CORES, NG, NRANGE, GB)
    maxc = counts.max(axis=0)
    tiles_grd = ((maxc + P - 1) // P).astype(np.int64)
    return tiles_grd, key, counts, grow, dloc


def _build_kernel(tiles_grd):
    nc = bacc.Bacc("TRN2", target_bir_lowering=False, debug=False,
                   num_devices=CORES)
    t_call = tiles_grd.sum(axis=2)
    t_group = t_call.sum(axis=1)
    g_start = np.concatenate([[0], np.cumsum(t_group)])
    NT_TOT = int(g_start[-1])
    T_MAX = int(t_group.max())

    xT_d = nc.dram_tensor("xT", [F_IN, NSH_PAD], F32, kind="ExternalInput")
    w1_d = nc.dram_tensor("W1", [F_IN, HID], F32, kind="ExternalInput")
    b1t_d = nc.dram_tensor("B1T", [P, HID], F32, kind="ExternalInput")
    w2cb_d = nc.dram_tensor("W2CB", [P, 2 * HID], F32, kind="ExternalInput")
    iota_d = nc.dram_tensor("IOTA", [P, P], F32, kind="ExternalInput")
    dinv_d = nc.dram_tensor("DINV", [P, NB], F32, kind="ExternalInput")
    idx_d = nc.dram_tensor("IDX", [P, 8 * NT_TOT], I16, kind="ExternalInput")
    dstloc_d = nc.dram_tensor("DSTLOC", [P, NT_TOT], F32, kind="ExternalInput")
    out_d = nc.dram_tensor("OUT", [P, 2 * NB], F32, kind="ExternalOutput")

    KA = min(F_IN, P)
    KB = F_IN - KA

    with tile.TileContext(nc) as tc:
        with (
            tc.tile_pool(name="const", bufs=1) as cpool,
            tc.tile_pool(name="dram", bufs=1, space="DRAM") as dpool,
        ):
            iota_sb = cpool.tile([P, P], F32)
            dinv_sb = cpool.tile([P, NB], F32)
            dstloc_sb = cpool.tile([P, NT_TOT], F32)
            b1t_sb = cpool.tile([P, HID], F32)
            w2cb_sb = cpool.tile([P, 2 * HID], F32)
            outsb = cpool.tile([P, 2 * NB], F32)
            nc.sync.dma_start(out=iota_sb[:], in_=iota_d[:, :])
            nc.sync.dma_start(out=dinv_sb[:], in_=dinv_d[:, :])
            nc.sync.dma_start(out=dstloc_sb[:], in_=dstloc_d[:, :])
            nc.sync.dma_start(out=b1t_sb[:], in_=b1t_d[:, :])
            nc.sync.dma_start(out=w2cb_sb[:], in_=w2cb_d[:, :])

            sh1 = dpool.tile([NSH_PAD, HID], BF16)
            tb1 = dpool.tile([V_PAD, HID], BF16)
            sh2 = dpool.tile([NSH_PAD, HID], BF16)
            tb2 = dpool.tile([V_PAD, HID], BF16)

            # Phase A: g1 = (x @ W1) * dinv -> sh1
            with (
                tc.tile_pool(name="mm_w", bufs=1) as wpool,
                tc.tile_pool(name="mm_x", bufs=1) as xpool,
                tc.tile_pool(name="mm_ps", bufs=4, space="PSUM") as pspool,
                tc.tile_pool(name="mm_g", bufs=4) as gpool,
            ):
                w1a = wpool.tile([KA, HID], F32)
                nc.sync.dma_start(out=w1a[:], in_=w1_d[0:KA, :])
                w1b = wpool.tile([KB, HID], F32)
                nc.sync.dma_start(out=w1b[:], in_=w1_d[KA:F_IN, :])
                xta = xpool.tile([KA, NSH_PAD], F32)
                nc.sync.dma_start(out=xta[:], in_=xT_d[0:KA, :])
                xtb = xpool.tile([KB, NSH_PAD], F32)
                nc.sync.dma_start(out=xtb[:], in_=xT_d[KA:F_IN, :])

                for d in range(NB):
                    ps = pspool.tile([P, HID], F32, space="PSUM", tag="ps_a")
                    sl = slice(d * P, (d + 1) * P)
                    nc.tensor.matmul(out=ps[:], lhsT=xta[:, sl], rhs=w1a[:],
                                     start=True, stop=False)
                    nc.tensor.matmul(out=ps[:], lhsT=xtb[:, sl], rhs=w1b[:],
                                     start=False, stop=True)
                    g1 = gpool.tile([P, HID], BF16, tag="g_a")
                    nc.vector.tensor_scalar(out=g1[:], in0=ps[:],
                                            scalar1=dinv_sb[:, d:d + 1],
                                            scalar2=None,
                                            op0=mybir.AluOpType.mult)
                    nc.sync.dma_start(out=sh1[sl, :], in_=g1[:])

            nc.gpsimd.collective_compute(
                "AllGather", mybir.AluOpType.bypass,
                replica_groups=[list(range(CORES))],
                ins=[sh1[:].opt()], outs=[tb1[:].opt()],
            )

            def agg_pass(table, layer):
                with (
                    tc.tile_pool(name=f"st{layer}", bufs=2) as stpool,
                    tc.tile_pool(name=f"ix{layer}", bufs=2) as ixpool,
                    tc.tile_pool(name=f"s{layer}", bufs=1) as spool,
                    tc.tile_pool(name=f"ps{layer}", bufs=4, space="PSUM") as pspool,
                    tc.tile_pool(name=f"z{layer}", bufs=4) as zpool,
                ):
                    for g in range(NG):
                        gs = int(g_start[g])
                        tg = int(t_group[g])
                        if tg == 0:
                            continue
                        stage = stpool.tile([P, T_MAX * HID], BF16, tag="stage")
                        ix = ixpool.tile([P, 8 * T_MAX], I16, tag="ix")
                        nc.sync.dma_start(out=ix[:, 0:8 * tg],
                                          in_=idx_d[:, 8 * gs:8 * (gs + tg)])
                        toff = 0
                        for r in range(NRANGE):
                            tc_r = int(t_call[g][r])
                            if tc_r == 0:
                                continue
                            K = tc_r * P
                            nc.gpsimd.dma_gather(
                                out_ap=stage[:, toff * HID:(toff + tc_r) * HID]
                                    .rearrange("p (t j) -> p t j", j=HID),
                                in_ap=table[r * RANGE:(r + 1) * RANGE, :],
                                idxs_ap=ix[:, 8 * toff:8 * (toff + tc_r)],
                                num_idxs=K, num_idxs_reg=K, elem_size=HID,
                                single_packet=False)
                            toff += tc_r
                        s_all = spool.tile([P, T_MAX * P], BF16, tag="s")
                        nc.vector.tensor_tensor(
                            out=s_all[:, 0:tg * P].rearrange(
                                "p (t j) -> p t j", j=P),
                            in0=iota_sb[:].unsqueeze(1).to_broadcast([P, tg, P]),
                            in1=dstloc_sb[:, gs:gs + tg].unsqueeze(2)
                                .to_broadcast([P, tg, P]),
                            op=mybir.AluOpType.is_equal)
                        for dl in range(GB):
                            d = g * GB + dl
                            njobs = int(tiles_grd[g, :, dl].sum())
                            if njobs == 0:
                                continue
                            ps = pspool.tile([P, HID], F32, space="PSUM",
                                             tag="ps")
                            done = 0
                            roff = 0
                            for r in range(NRANGE):
                                base = roff + int(tiles_grd[g, r, :dl].sum())
                                for t in range(int(tiles_grd[g, r, dl])):
                                    c = base + t
                                    nc.tensor.matmul(
                                        out=ps[:],
                                        lhsT=s_all[:, c * P:(c + 1) * P],
                                        rhs=stage[:, c * HID:(c + 1) * HID],
                                        start=(done == 0),
                                        stop=(done == njobs - 1))
                                    done += 1
                                roff += int(t_call[g][r])
                            yield d, ps, zpool

            # AGG1 + layer-1 tail
            with tc.tile_pool(name="pb1", bufs=4) as pbpool:
                for d, ps, zpool in agg_pass(tb1, 1):
                    z = zpool.tile([P, HID], F32, tag="z1")
                    nc.vector.tensor_scalar(out=z[:], in0=ps[:],
                                            scalar1=dinv_sb[:, d:d + 1],
                                            scalar2=None,
                                            op0=mybir.AluOpType.mult)
                    nc.vector.tensor_tensor(out=z[:], in0=z[:], in1=b1t_sb[:],
                                            op=mybir.AluOpType.add)
                    p_bf = pbpool.tile([P, HID], BF16, tag="pb")
                    nc.scalar.activation(out=p_bf[:], in_=z[:],
                                         func=mybir.ActivationFunctionType.Relu,
                                         scale=dinv_sb[:, d:d + 1])
                    nc.sync.dma_start(out=sh2[d * P:(d + 1) * P, :], in_=p_bf[:])

            nc.gpsimd.collective_compute(
                "AllGather", mybir.AluOpType.bypass,
                replica_groups=[list(range(CORES))],
                ins=[sh2[:].opt()], outs=[tb2[:].opt()],
            )

            # AGG2 + layer-2 tail
            nc.vector.memset(outsb[:], 0.0)
            with tc.tile_pool(name="tmp2", bufs=4) as tmppool:
                for d, ps, zpool in agg_pass(tb2, 2):
                    z = zpool.tile([P, HID], F32, tag="z2")
                    nc.vector.tensor_scalar(out=z[:], in0=ps[:],
                                            scalar1=dinv_sb[:, d:d + 1],
                                            scalar2=None,
                                            op0=mybir.AluOpType.mult)
                    for ch in range(2):
                        tmp = tmppool.tile([P, HID], F32, tag="t2")
                        nc.vector.tensor_tensor(
                            out=tmp[:], in0=z[:],
                            in1=w2cb_sb[:, ch * HID:(ch + 1) * HID],
                            op=mybir.AluOpType.mult)
                        nc.vector.tensor_reduce(
                            out=outsb[:, 2 * d + ch:2 * d + ch + 1],
                            in_=tmp[:], axis=mybir.AxisListType.X,
                            op=mybir.AluOpType.add)

            nc.sync.dma_start(out=out_d[:, :], in_=outsb[:])

    nc.compile()
    return nc


def _prep(x, edge_index, W1, b1, W2):
    src = np.asarray(edge_index[0], dtype=np.int64)
    dst = np.asarray(edge_index[1], dtype=np.int64)
    loop = np.arange(N_NODES, dtype=np.int64)
    src = np.concatenate([src, loop])
    dst = np.concatenate([dst, loop])

    deg = np.bincount(dst, minlength=N_NODES)
    dinv = np.where(deg > 0, 1.0 / np.sqrt(deg.astype(np.float64)),
                    0.0).astype(np.float32)

    tiles_grd, key, counts, grow, dloc = _edge_structure(src, dst)

    t_call = tiles_grd.sum(axis=2)
    t_group = t_call.sum(axis=1)
    g_start = np.concatenate([[0], np.cumsum(t_group)])
    NT_TOT = int(g_start[-1])

    call_base = np.zeros((NG, NRANGE), np.int64)
    for g in range(NG):
        acc = int(g_start[g])
        for r in range(NRANGE):
            call_base[g, r] = acc
            acc += int(t_call[g, r])
    buck_base = np.zeros((NG, NRANGE, GB), np.int64)
    for g in range(NG):
        for r in range(NRANGE):
            acc = 0
            for dl in range(GB):
                buck_base[g, r, dl] = acc
                acc += int(tiles_grd[g, r, dl])

    order = np.argsort(key, kind="stable")
    starts = np.concatenate([[0], np.cumsum(counts.reshape(-1))])

    idx_all = np.zeros((CORES, P, 8 * NT_TOT), np.int16)
    dst_all = np.full((CORES, P, NT_TOT), -1.0, np.float32)
    kflat = 0
    for c in range(CORES):
        for g in range(NG):
            for r in range(NRANGE):
                for dl in range(GB):
                    s0, s1 = starts[kflat], starts[kflat + 1]
                    kflat += 1
                    cnt = s1 - s0
                    if cnt == 0:
                        continue
                    e = order[s0:s1]
                    J = buck_base[g, r, dl] * P + np.arange(cnt)
                    gt = call_base[g, r] + J // P
                    pp = J % P
                    dst_all[c, pp, gt] = dloc[e].astype(np.float32)
                    icol = 8 * call_base[g, r] + J // 16
                    idx_all[c, J % 16, icol] = (grow[e] - r * RANGE).astype(
                        np.int16)
    for q in range(1, 8):
        idx_all[:, 16 * q:16 * (q + 1), :] = idx_all[:, 0:16, :]

    dinv_pb = np.zeros((CORES, P, NB), np.float32)
    for c in range(CORES):
        n0, n1 = c * NSH, min((c + 1) * NSH, N_NODES)
        loc = np.zeros(NSH_PAD, np.float32)
        loc[: n1 - n0] = dinv[n0:n1]
        dinv_pb[c] = loc.reshape(NB, P).T

    xT = np.ascontiguousarray(np.asarray(x, np.float32).T)
    b1t = np.tile(np.asarray(b1, np.float32)[None, :], (P, 1))
    w2 = np.asarray(W2, np.float32)
    w2cb = np.zeros((P, 2 * HID), np.float32)
    for ch in range(2):
        w2cb[:, ch * HID:(ch + 1) * HID] = np.tile(w2[:, ch][None, :], (P, 1))
    iota = np.tile(np.arange(P, dtype=np.float32)[None, :], (P, 1))
    W1f = np.asarray(W1, np.float32)

    in_maps = []
    for c in range(CORES):
        n0, n1 = c * NSH, min((c + 1) * NSH, N_NODES)
        xtc = np.zeros((F_IN, NSH_PAD), np.float32)
        xtc[:, : n1 - n0] = xT[:, n0:n1]
        in_maps.append({
            "xT": xtc, "W1": W1f, "B1T": b1t, "W2CB": w2cb, "IOTA": iota,
            "DINV": dinv_pb[c], "IDX": idx_all[c], "DSTLOC": dst_all[c],
        })
    return tiles_grd, in_maps


def kernel(x, edge_index, W1, b1, W2, b2):
    import os
    x = np.asarray(x)
    edge_index = np.asarray(edge_index)
    W1 = np.asarray(W1)
    b1 = np.asarray(b1)
    W2 = np.asarray(W2)
    b2 = np.asarray(b2, dtype=np.float32)
    assert x.shape == (N_NODES, F_IN), x.shape

    tiles_grd, in_maps = _prep(x, edge_index, W1, b1, W2)
    nc = _build_kernel(tiles_grd)
    trace = bool(int(os.environ.get("GCN_TRACE", "0")))
    try:
        res = run_bass_kernel_spmd(nc, in_maps, core_ids=list(range(CORES)),
                                   trace=trace)
    except Exception:
        if not trace:
            raise
        res = run_bass_kernel_spmd(nc, in_maps, core_ids=list(range(CORES)),
                                   trace=False)
    if trace and res.exec_time_ns is not None:
        print(f"HW exec time: {res.exec_time_ns} ns")

    out = np.zeros((N_NODES, 2), np.float32)
    for c in range(CORES):
        buf = res.results[c]["OUT"]
        arr = buf.reshape(P, NB, 2).transpose(1, 0, 2).reshape(NSH_PAD, 2)
        n0, n1 = c * NSH, min((c + 1) * NSH, N_NODES)
        out[n0:n1] = arr[: n1 - n0]
    return out + b2[None, :]



# revision 5
# speedup vs baseline: 1.7458x; 1.7458x over previous
"""2-layer GCN (PyG GCNConv x2, eval mode) on 8 TRN2 NeuronCores, SPMD.

Strategy (graph-partition data parallel, per the sharding hint):
  - Nodes are sharded contiguously across the 8 cores (dst ownership).
  - Per layer:  t = X @ W ; g = t * dinv  (D^-1/2 factored per-node),
    g is cast to bf16 and AllGathered so every core holds the full node table;
    each core then aggregates its owned destinations:
        q[v] = sum_{e: dst=v} g[src]   (self loops are host-added edges)
    and applies the tail (z = q*dinv + b, relu for layer 1).
  - Aggregation engine: edges are bucketed host-side by
    (dst-block of 128, src range of V/4) so the custom `dma_gather`
    (int16 indices, 256B bf16 rows) can pull message rows into SBUF
    partition-major tiles; the DVE builds a one-hot S = (iota == dst_local)
    per 128-edge tile and the PE accumulates S^T @ rows into PSUM per dst
    block (segment-sum as matmul).  Pad slots gather row 0 and carry
    dst_local -1 (zero one-hot column), so they contribute nothing.
  - Layer 2 output (C=2) is formed as two masked row-reductions on the DVE;
    b2 is added on the host.
"""

import numpy as np
import ml_dtypes

import concourse.bass as bass
import concourse.mybir as mybir
import concourse.tile as tile
import concourse.bacc as bacc
from concourse.bass_utils import run_bass_kernel_spmd

F32 = mybir.dt.float32
BF16 = mybir.dt.bfloat16
I16 = mybir.dt.int16

HID = 128
P = 128
NRANGE = 4
N_NODES = 100000
F_IN = 165
CORES = 8
NB = 100          # dst blocks per core (12800 padded nodes/core)
GB = 5            # dst blocks per gather group
NSH = 12500
NSH_PAD = NB * P
V_PAD = CORES * NSH_PAD
RANGE = V_PAD // NRANGE
NG = NB // GB


def _edge_structure(src, dst):
    csh = np.minimum(dst // NSH, CORES - 1)
    r_loc = dst - csh * NSH
    blk = r_loc // P
    dloc = r_loc % P
    csrc = np.minimum(src // NSH, CORES - 1)
    grow = csrc * NSH_PAD + (src - csrc * NSH)
    rng = grow // RANGE
    grp = blk // GB
    dl = blk % GB
    key = ((csh * NG + grp) * NRANGE + rng) * GB + dl
    nkeys = CORES * NG * NRANGE * GB
    counts = np.bincount(key, minlength=nkeys).reshape(CORES, NG, NRANGE, GB)
    maxc = counts.max(axis=0)
    tiles_grd = ((maxc + P - 1) // P).astype(np.int64)
    return tiles_grd, key, counts, grow, dloc


def _build_kernel(tiles_grd):
    nc = bacc.Bacc("TRN2", target_bir_lowering=False, debug=False,
                   num_devices=CORES, num_swdge_queues=4)
    t_call = tiles_grd.sum(axis=2)
    t_group = t_call.sum(axis=1)
    g_start = np.concatenate([[0], np.cumsum(t_group)])
    NT_TOT = int(g_start[-1])
    T_MAX = int(t_group.max())

    xT_d = nc.dram_tensor("xT", [F_IN, NSH_PAD], BF16, kind="ExternalInput")
    w1_d = nc.dram_tensor("W1", [F_IN, HID], BF16, kind="ExternalInput")
    b1t_d = nc.dram_tensor("B1T", [P, HID], F32, kind="ExternalInput")
    w2cb_d = nc.dram_tensor("W2CB", [P, 2 * HID], F32, kind="ExternalInput")
    iota_d = nc.dram_tensor("IOTA", [P, P], F32, kind="ExternalInput")
    dinv_d = nc.dram_tensor("DINV", [P, NB], F32, kind="ExternalInput")
    idx_d = nc.dram_tensor("IDX", [P, 8 * NT_TOT], I16, kind="ExternalInput")
    dstloc_d = nc.dram_tensor("DSTLOC", [P, NT_TOT], F32, kind="ExternalInput")
    out_d = nc.dram_tensor("OUT", [P, 2 * NB], F32, kind="ExternalOutput")

    KA = min(F_IN, P)
    KB = F_IN - KA

    with tile.TileContext(nc) as tc:
        with (
            tc.tile_pool(name="const", bufs=1) as cpool,
            tc.tile_pool(name="dram", bufs=1, space="DRAM") as dpool,
        ):
            iota_sb = cpool.tile([P, P], F32)
            dinv_sb = cpool.tile([P, NB], F32)
            dstloc_sb = cpool.tile([P, NT_TOT], F32)
            b1t_sb = cpool.tile([P, HID], F32)
            w2cb_sb = cpool.tile([P, 2 * HID], F32)
            outsb = cpool.tile([P, 2 * NB], F32)
            nc.sync.dma_start(out=iota_sb[:], in_=iota_d[:, :])
            nc.sync.dma_start(out=dinv_sb[:], in_=dinv_d[:, :])
            nc.sync.dma_start(out=dstloc_sb[:], in_=dstloc_d[:, :])
            nc.sync.dma_start(out=b1t_sb[:], in_=b1t_d[:, :])
            nc.sync.dma_start(out=w2cb_sb[:], in_=w2cb_d[:, :])

            sh1 = dpool.tile([NSH_PAD, HID], BF16)
            tb1 = dpool.tile([V_PAD, HID], BF16)
            sh2 = dpool.tile([NSH_PAD, HID], BF16)
            tb2 = dpool.tile([V_PAD, HID], BF16)

            # Phase A: g1 = (x @ W1) * dinv -> sh1
            with (
                tc.tile_pool(name="mm_w", bufs=1) as wpool,
                tc.tile_pool(name="mm_x", bufs=1) as xpool,
                tc.tile_pool(name="mm_ps", bufs=4, space="PSUM") as pspool,
                tc.tile_pool(name="mm_g", bufs=4) as gpool,
            ):
                w1a = wpool.tile([KA, HID], BF16)
                nc.sync.dma_start(out=w1a[:], in_=w1_d[0:KA, :])
                w1b = wpool.tile([KB, HID], BF16)
                nc.sync.dma_start(out=w1b[:], in_=w1_d[KA:F_IN, :])
                xta = xpool.tile([KA, NSH_PAD], BF16)
                nc.sync.dma_start(out=xta[:], in_=xT_d[0:KA, :])
                xtb = xpool.tile([KB, NSH_PAD], BF16)
                nc.sync.dma_start(out=xtb[:], in_=xT_d[KA:F_IN, :])

                for d in range(NB):
                    ps = pspool.tile([P, HID], F32, space="PSUM", tag="ps_a")
                    sl = slice(d * P, (d + 1) * P)
                    nc.tensor.matmul(out=ps[:], lhsT=xta[:, sl], rhs=w1a[:],
                                     start=True, stop=False)
                    nc.tensor.matmul(out=ps[:], lhsT=xtb[:, sl], rhs=w1b[:],
                                     start=False, stop=True)
                    g1 = gpool.tile([P, HID], BF16, tag="g_a")
                    nc.vector.tensor_scalar(out=g1[:], in0=ps[:],
                                            scalar1=dinv_sb[:, d:d + 1],
                                            scalar2=None,
                                            op0=mybir.AluOpType.mult)
                    nc.sync.dma_start(out=sh1[sl, :], in_=g1[:])

            nc.gpsimd.collective_compute(
                "AllGather", mybir.AluOpType.bypass,
                replica_groups=[list(range(CORES))],
                ins=[sh1[:].opt()], outs=[tb1[:].opt()],
            )

            def agg_pass(table, layer):
                with (
                    tc.tile_pool(name=f"st{layer}", bufs=2) as stpool,
                    tc.tile_pool(name=f"ix{layer}", bufs=2) as ixpool,
                    tc.tile_pool(name=f"s{layer}", bufs=1) as spool,
                    tc.tile_pool(name=f"ps{layer}", bufs=4, space="PSUM") as pspool,
                    tc.tile_pool(name=f"z{layer}", bufs=4) as zpool,
                ):
                    for g in range(NG):
                        gs = int(g_start[g])
                        tg = int(t_group[g])
                        if tg == 0:
                            continue
                        stage = stpool.tile([P, T_MAX * HID], BF16, tag="stage")
                        ix = ixpool.tile([P, 8 * T_MAX], I16, tag="ix")
                        nc.sync.dma_start(out=ix[:, 0:8 * tg],
                                          in_=idx_d[:, 8 * gs:8 * (gs + tg)])
                        toff = 0
                        for r in range(NRANGE):
                            tc_r = int(t_call[g][r])
                            if tc_r == 0:
                                continue
                            K = tc_r * P
                            nc.gpsimd.dma_gather(
                                out_ap=stage[:, toff * HID:(toff + tc_r) * HID]
                                    .rearrange("p (t j) -> p t j", j=HID),
                                in_ap=table[r * RANGE:(r + 1) * RANGE, :],
                                idxs_ap=ix[:, 8 * toff:8 * (toff + tc_r)],
                                num_idxs=K, num_idxs_reg=K, elem_size=HID,
                                single_packet=False, queue_num=1 + r % 3)
                            toff += tc_r
                        s_all = spool.tile([P, T_MAX * P], BF16, tag="s")
                        nc.vector.tensor_tensor(
                            out=s_all[:, 0:tg * P].rearrange(
                                "p (t j) -> p t j", j=P),
                            in0=iota_sb[:].unsqueeze(1).to_broadcast([P, tg, P]),
                            in1=dstloc_sb[:, gs:gs + tg].unsqueeze(2)
                                .to_broadcast([P, tg, P]),
                            op=mybir.AluOpType.is_equal)
                        for dl in range(GB):
                            d = g * GB + dl
                            njobs = int(tiles_grd[g, :, dl].sum())
                            if njobs == 0:
                                continue
                            ps = pspool.tile([P, HID], F32, space="PSUM",
                                             tag="ps")
                            done = 0
                            roff = 0
                            for r in range(NRANGE):
                                base = roff + int(tiles_grd[g, r, :dl].sum())
                                for t in range(int(tiles_grd[g, r, dl])):
                                    c = base + t
                                    nc.tensor.matmul(
                                        out=ps[:],
                                        lhsT=s_all[:, c * P:(c + 1) * P],
                                        rhs=stage[:, c * HID:(c + 1) * HID],
                                        start=(done == 0),
                                        stop=(done == njobs - 1))
                                    done += 1
                                roff += int(t_call[g][r])
                            yield d, ps, zpool

            # AGG1 + layer-1 tail
            with tc.tile_pool(name="pb1", bufs=4) as pbpool:
                for d, ps, zpool in agg_pass(tb1, 1):
                    z = zpool.tile([P, HID], F32, tag="z1")
                    nc.vector.tensor_scalar(out=z[:], in0=ps[:],
                                            scalar1=dinv_sb[:, d:d + 1],
                                            scalar2=None,
                                            op0=mybir.AluOpType.mult)
                    nc.vector.tensor_tensor(out=z[:], in0=z[:], in1=b1t_sb[:],
                                            op=mybir.AluOpType.add)
                    p_bf = pbpool.tile([P, HID], BF16, tag="pb")
                    nc.scalar.activation(out=p_bf[:], in_=z[:],
                                         func=mybir.ActivationFunctionType.Relu,
                                         scale=dinv_sb[:, d:d + 1])
                    nc.sync.dma_start(out=sh2[d * P:(d + 1) * P, :], in_=p_bf[:])

            nc.gpsimd.collective_compute(
                "AllGather", mybir.AluOpType.bypass,
                replica_groups=[list(range(CORES))],
                ins=[sh2[:].opt()], outs=[tb2[:].opt()],
            )

            # AGG2 + layer-2 tail
            nc.vector.memset(outsb[:], 0.0)
            with tc.tile_pool(name="tmp2", bufs=4) as tmppool:
                for d, ps, zpool in agg_pass(tb2, 2):
                    z = zpool.tile([P, HID], F32, tag="z2")
                    nc.vector.tensor_scalar(out=z[:], in0=ps[:],
                                            scalar1=dinv_sb[:, d:d + 1],
                                            scalar2=None,
                                            op0=mybir.AluOpType.mult)
                    for ch in range(2):
                        tmp = tmppool.tile([P, HID], F32, tag="t2")
                        nc.vector.tensor_tensor(
                            out=tmp[:], in0=z[:],
                            in1=w2cb_sb[:, ch * HID:(ch + 1) * HID],
                            op=mybir.AluOpType.mult)
                        nc.vector.tensor_reduce(
                            out=outsb[:, 2 * d + ch:2 * d + ch + 1],
                            in_=tmp[:], axis=mybir.AxisListType.X,
                            op=mybir.AluOpType.add)

            nc.sync.dma_start(out=out_d[:, :], in_=outsb[:])

    nc.compile()
    return nc


def _prep(x, edge_index, W1, b1, W2):
    src = np.asarray(edge_index[0], dtype=np.int64)
    dst = np.asarray(edge_index[1], dtype=np.int64)
    loop = np.arange(N_NODES, dtype=np.int64)
    src = np.concatenate([src, loop])
    dst = np.concatenate([dst, loop])

    deg = np.bincount(dst, minlength=N_NODES)
    dinv = np.where(deg > 0, 1.0 / np.sqrt(deg.astype(np.float64)),
                    0.0).astype(np.float32)

    tiles_grd, key, counts, grow, dloc = _edge_structure(src, dst)

    t_call = tiles_grd.sum(axis=2)
    t_group = t_call.sum(axis=1)
    g_start = np.concatenate([[0], np.cumsum(t_group)])
    NT_TOT = int(g_start[-1])

    call_base = np.zeros((NG, NRANGE), np.int64)
    for g in range(NG):
        acc = int(g_start[g])
        for r in range(NRANGE):
            call_base[g, r] = acc
            acc += int(t_call[g, r])
    buck_base = np.zeros((NG, NRANGE, GB), np.int64)
    for g in range(NG):
        for r in range(NRANGE):
            acc = 0
            for dl in range(GB):
                buck_base[g, r, dl] = acc
                acc += int(tiles_grd[g, r, dl])

    order = np.argsort(key, kind="stable")
    starts = np.concatenate([[0], np.cumsum(counts.reshape(-1))])

    idx_all = np.zeros((CORES, P, 8 * NT_TOT), np.int16)
    dst_all = np.full((CORES, P, NT_TOT), -1.0, np.float32)
    kflat = 0
    for c in range(CORES):
        for g in range(NG):
            for r in range(NRANGE):
                for dl in range(GB):
                    s0, s1 = starts[kflat], starts[kflat + 1]
                    kflat += 1
                    cnt = s1 - s0
                    if cnt == 0:
                        continue
                    e = order[s0:s1]
                    J = buck_base[g, r, dl] * P + np.arange(cnt)
                    gt = call_base[g, r] + J // P
                    pp = J % P
                    dst_all[c, pp, gt] = dloc[e].astype(np.float32)
                    icol = 8 * call_base[g, r] + J // 16
                    idx_all[c, J % 16, icol] = (grow[e] - r * RANGE).astype(
                        np.int16)
    for q in range(1, 8):
        idx_all[:, 16 * q:16 * (q + 1), :] = idx_all[:, 0:16, :]

    dinv_pb = np.zeros((CORES, P, NB), np.float32)
    for c in range(CORES):
        n0, n1 = c * NSH, min((c + 1) * NSH, N_NODES)
        loc = np.zeros(NSH_PAD, np.float32)
        loc[: n1 - n0] = dinv[n0:n1]
        dinv_pb[c] = loc.reshape(NB, P).T

    xT = np.ascontiguousarray(np.asarray(x, np.float32).T.astype(ml_dtypes.bfloat16))
    b1t = np.tile(np.asarray(b1, np.float32)[None, :], (P, 1))
    w2 = np.asarray(W2, np.float32)
    w2cb = np.zeros((P, 2 * HID), np.float32)
    for ch in range(2):
        w2cb[:, ch * HID:(ch + 1) * HID] = np.tile(w2[:, ch][None, :], (P, 1))
    iota = np.tile(np.arange(P, dtype=np.float32)[None, :], (P, 1))
    W1f = np.asarray(W1, np.float32).astype(ml_dtypes.bfloat16)

    in_maps = []
    for c in range(CORES):
        n0, n1 = c * NSH, min((c + 1) * NSH, N_NODES)
        xtc = np.zeros((F_IN, NSH_PAD), ml_dtypes.bfloat16)
        xtc[:, : n1 - n0] = xT[:, n0:n1]
        in_maps.append({
            "xT": xtc, "W1": W1f, "B1T": b1t, "W2CB": w2cb, "IOTA": iota,
            "DINV": dinv_pb[c], "IDX": idx_all[c], "DSTLOC": dst_all[c],
        })
    return tiles_grd, in_maps


def kernel(x, edge_index, W1, b1, W2, b2):
    import os
    x = np.asarray(x)
    edge_index = np.asarray(edge_index)
    W1 = np.asarray(W1)
    b1 = np.asarray(b1)
    W2 = np.asarray(W2)
    b2 = np.asarray(b2, dtype=np.float32)
    assert x.shape == (N_NODES, F_IN), x.shape

    tiles_grd, in_maps = _prep(x, edge_index, W1, b1, W2)
    nc = _build_kernel(tiles_grd)
    trace = bool(int(os.environ.get("GCN_TRACE", "0")))
    try:
        res = run_bass_kernel_spmd(nc, in_maps, core_ids=list(range(CORES)),
                                   trace=trace)
    except Exception:
        if not trace:
            raise
        res = run_bass_kernel_spmd(nc, in_maps, core_ids=list(range(CORES)),
                                   trace=False)
    if trace and res.exec_time_ns is not None:
        print(f"HW exec time: {res.exec_time_ns} ns")

    out = np.zeros((N_NODES, 2), np.float32)
    for c in range(CORES):
        buf = res.results[c]["OUT"]
        arr = buf.reshape(P, NB, 2).transpose(1, 0, 2).reshape(NSH_PAD, 2)
        n0, n1 = c * NSH, min((c + 1) * NSH, N_NODES)
        out[n0:n1] = arr[: n1 - n0]
    return out + b2[None, :]



# revision 6
# speedup vs baseline: 2.0808x; 1.1919x over previous
"""2-layer GCN (PyG GCNConv x2, eval mode) on 8 TRN2 NeuronCores, SPMD.

Strategy (graph-partition data parallel, per the sharding hint):
  - Nodes are sharded contiguously across the 8 cores (dst ownership).
  - Per layer:  t = X @ W ; g = t * dinv  (D^-1/2 factored per-node),
    g is cast to bf16 and AllGathered so every core holds the full node table;
    each core then aggregates its owned destinations:
        q[v] = sum_{e: dst=v} g[src]   (self loops are host-added edges)
    and applies the tail (z = q*dinv + b, relu for layer 1).
  - Aggregation engine: edges are bucketed host-side by
    (dst-block of 128, src range of V/4) so the custom `dma_gather`
    (int16 indices, 256B bf16 rows) can pull message rows into SBUF
    partition-major tiles; the DVE builds a one-hot S = (iota == dst_local)
    per 128-edge tile and the PE accumulates S^T @ rows into PSUM per dst
    block (segment-sum as matmul).  Pad slots gather row 0 and carry
    dst_local -1 (zero one-hot column), so they contribute nothing.
  - Layer 2 output (C=2) is formed as two masked row-reductions on the DVE;
    b2 is added on the host.
"""

import numpy as np
import ml_dtypes

import concourse.bass as bass
import concourse.mybir as mybir
import concourse.tile as tile
import concourse.bacc as bacc
from concourse.bass_utils import run_bass_kernel_spmd

F32 = mybir.dt.float32
BF16 = mybir.dt.bfloat16
I16 = mybir.dt.int16

HID = 128
P = 128
NRANGE = 4
N_NODES = 100000
F_IN = 165
CORES = 8
NB = 100          # dst blocks per core (12800 padded nodes/core)
GB = 5            # dst blocks per gather group
NSH = 12500
NSH_PAD = NB * P
V_PAD = CORES * NSH_PAD
RANGE = V_PAD // NRANGE
NG = NB // GB


def _edge_structure(src, dst):
    csh = np.minimum(dst // NSH, CORES - 1)
    r_loc = dst - csh * NSH
    blk = r_loc // P
    dloc = r_loc % P
    csrc = np.minimum(src // NSH, CORES - 1)
    grow = csrc * NSH_PAD + (src - csrc * NSH)
    rng = grow // RANGE
    grp = blk // GB
    dl = blk % GB
    key = ((csh * NG + grp) * NRANGE + rng) * GB + dl
    nkeys = CORES * NG * NRANGE * GB
    counts = np.bincount(key, minlength=nkeys).reshape(CORES, NG, NRANGE, GB)
    maxc = counts.max(axis=0)
    tiles_grd = ((maxc + P - 1) // P).astype(np.int64)
    return tiles_grd, key, counts, grow, dloc


def _build_kernel(tiles_grd):
    nc = bacc.Bacc("TRN2", target_bir_lowering=False, debug=False,
                   num_devices=CORES, num_swdge_queues=4)
    t_call = tiles_grd.sum(axis=2)
    t_group = t_call.sum(axis=1)
    g_start = np.concatenate([[0], np.cumsum(t_group)])
    NT_TOT = int(g_start[-1])
    T_MAX = int(t_group.max())

    xT_d = nc.dram_tensor("xT", [F_IN, NSH_PAD], BF16, kind="ExternalInput")
    w1_d = nc.dram_tensor("W1", [F_IN, HID], BF16, kind="ExternalInput")
    b1t_d = nc.dram_tensor("B1T", [P, HID], F32, kind="ExternalInput")
    w2cb_d = nc.dram_tensor("W2CB", [P, 2 * HID], F32, kind="ExternalInput")
    iota_d = nc.dram_tensor("IOTA", [P, P], F32, kind="ExternalInput")
    dinv_d = nc.dram_tensor("DINV", [P, NB], F32, kind="ExternalInput")
    idx_d = nc.dram_tensor("IDX", [P, 8 * NT_TOT], I16, kind="ExternalInput")
    dstloc_d = nc.dram_tensor("DSTLOC", [P, NT_TOT], F32, kind="ExternalInput")
    out_d = nc.dram_tensor("OUT", [P, 2 * NB], F32, kind="ExternalOutput")

    KA = min(F_IN, P)
    KB = F_IN - KA

    with tile.TileContext(nc) as tc:
        with (
            tc.tile_pool(name="const", bufs=1) as cpool,
            tc.tile_pool(name="dram", bufs=1, space="DRAM") as dpool,
        ):
            iota_sb = cpool.tile([P, P], F32)
            dinv_sb = cpool.tile([P, NB], F32)
            dstloc_sb = cpool.tile([P, NT_TOT], F32)
            b1t_sb = cpool.tile([P, HID], F32)
            w2cb_sb = cpool.tile([P, 2 * HID], F32)
            outsb = cpool.tile([P, 2 * NB], F32)
            nc.sync.dma_start(out=iota_sb[:], in_=iota_d[:, :])
            nc.sync.dma_start(out=dinv_sb[:], in_=dinv_d[:, :])
            nc.sync.dma_start(out=dstloc_sb[:], in_=dstloc_d[:, :])
            nc.sync.dma_start(out=b1t_sb[:], in_=b1t_d[:, :])
            nc.sync.dma_start(out=w2cb_sb[:], in_=w2cb_d[:, :])

            sh1 = dpool.tile([NSH_PAD, HID], BF16)
            tb1 = dpool.tile([V_PAD, HID], BF16)
            sh2 = dpool.tile([NSH_PAD, HID], BF16)
            tb2 = dpool.tile([V_PAD, HID], BF16)

            # Phase A: g1 = (x @ W1) * dinv -> sh1
            with (
                tc.tile_pool(name="mm_w", bufs=1) as wpool,
                tc.tile_pool(name="mm_x", bufs=1) as xpool,
                tc.tile_pool(name="mm_ps", bufs=4, space="PSUM") as pspool,
                tc.tile_pool(name="mm_g", bufs=4) as gpool,
            ):
                w1a = wpool.tile([KA, HID], BF16)
                nc.sync.dma_start(out=w1a[:], in_=w1_d[0:KA, :])
                w1b = wpool.tile([KB, HID], BF16)
                nc.sync.dma_start(out=w1b[:], in_=w1_d[KA:F_IN, :])
                xta = xpool.tile([KA, NSH_PAD], BF16)
                nc.sync.dma_start(out=xta[:], in_=xT_d[0:KA, :])
                xtb = xpool.tile([KB, NSH_PAD], BF16)
                nc.sync.dma_start(out=xtb[:], in_=xT_d[KA:F_IN, :])

                for d in range(NB):
                    ps = pspool.tile([P, HID], F32, space="PSUM", tag="ps_a")
                    sl = slice(d * P, (d + 1) * P)
                    nc.tensor.matmul(out=ps[:], lhsT=xta[:, sl], rhs=w1a[:],
                                     start=True, stop=False)
                    nc.tensor.matmul(out=ps[:], lhsT=xtb[:, sl], rhs=w1b[:],
                                     start=False, stop=True)
                    g1 = gpool.tile([P, HID], BF16, tag="g_a")
                    nc.vector.tensor_scalar(out=g1[:], in0=ps[:],
                                            scalar1=dinv_sb[:, d:d + 1],
                                            scalar2=None,
                                            op0=mybir.AluOpType.mult)
                    nc.sync.dma_start(out=sh1[sl, :], in_=g1[:])

            nc.gpsimd.collective_compute(
                "AllGather", mybir.AluOpType.bypass,
                replica_groups=[list(range(CORES))],
                ins=[sh1[:].opt()], outs=[tb1[:].opt()],
            )

            def agg_pass(table, layer):
                with (
                    tc.tile_pool(name=f"st{layer}", bufs=2) as stpool,
                    tc.tile_pool(name=f"ix{layer}", bufs=2) as ixpool,
                    tc.tile_pool(name=f"s{layer}", bufs=1) as spool,
                    tc.tile_pool(name=f"ps{layer}", bufs=4, space="PSUM") as pspool,
                    tc.tile_pool(name=f"z{layer}", bufs=4) as zpool,
                ):
                    for g in range(NG):
                        gs = int(g_start[g])
                        tg = int(t_group[g])
                        if tg == 0:
                            continue
                        stage = stpool.tile([P, T_MAX * HID], BF16, tag="stage")
                        ix = ixpool.tile([P, 8 * T_MAX], I16, tag="ix")
                        nc.sync.dma_start(out=ix[:, 0:8 * tg],
                                          in_=idx_d[:, 8 * gs:8 * (gs + tg)])
                        toff = 0
                        for r in range(NRANGE):
                            tc_r = int(t_call[g][r])
                            if tc_r == 0:
                                continue
                            K = tc_r * P
                            nc.gpsimd.dma_gather(
                                out_ap=stage[:, toff * HID:(toff + tc_r) * HID]
                                    .rearrange("p (t j) -> p t j", j=HID),
                                in_ap=table[r * RANGE:(r + 1) * RANGE, :],
                                idxs_ap=ix[:, 8 * toff:8 * (toff + tc_r)],
                                num_idxs=K, num_idxs_reg=K, elem_size=HID,
                                single_packet=False, queue_num=1 + (r + g) % 3)
                            toff += tc_r
                        s_all = spool.tile([P, T_MAX * P], BF16, tag="s")
                        nc.vector.tensor_tensor(
                            out=s_all[:, 0:tg * P].rearrange(
                                "p (t j) -> p t j", j=P),
                            in0=iota_sb[:].unsqueeze(1).to_broadcast([P, tg, P]),
                            in1=dstloc_sb[:, gs:gs + tg].unsqueeze(2)
                                .to_broadcast([P, tg, P]),
                            op=mybir.AluOpType.is_equal)
                        for dl in range(GB):
                            d = g * GB + dl
                            njobs = int(tiles_grd[g, :, dl].sum())
                            if njobs == 0:
                                continue
                            ps = pspool.tile([P, HID], F32, space="PSUM",
                                             tag="ps")
                            done = 0
                            roff = 0
                            for r in range(NRANGE):
                                base = roff + int(tiles_grd[g, r, :dl].sum())
                                for t in range(int(tiles_grd[g, r, dl])):
                                    c = base + t
                                    nc.tensor.matmul(
                                        out=ps[:],
                                        lhsT=s_all[:, c * P:(c + 1) * P],
                                        rhs=stage[:, c * HID:(c + 1) * HID],
                                        start=(done == 0),
                                        stop=(done == njobs - 1))
                                    done += 1
                                roff += int(t_call[g][r])
                            yield d, ps, zpool

            # AGG1 + layer-1 tail
            with tc.tile_pool(name="pb1", bufs=4) as pbpool:
                for d, ps, zpool in agg_pass(tb1, 1):
                    z = zpool.tile([P, HID], F32, tag="z1")
                    nc.vector.tensor_scalar(out=z[:], in0=ps[:],
                                            scalar1=dinv_sb[:, d:d + 1],
                                            scalar2=None,
                                            op0=mybir.AluOpType.mult)
                    nc.vector.tensor_tensor(out=z[:], in0=z[:], in1=b1t_sb[:],
                                            op=mybir.AluOpType.add)
                    p_bf = pbpool.tile([P, HID], BF16, tag="pb")
                    nc.scalar.activation(out=p_bf[:], in_=z[:],
                                         func=mybir.ActivationFunctionType.Relu,
                                         scale=dinv_sb[:, d:d + 1])
                    nc.sync.dma_start(out=sh2[d * P:(d + 1) * P, :], in_=p_bf[:])

            nc.gpsimd.collective_compute(
                "AllGather", mybir.AluOpType.bypass,
                replica_groups=[list(range(CORES))],
                ins=[sh2[:].opt()], outs=[tb2[:].opt()],
            )

            # AGG2 + layer-2 tail
            nc.vector.memset(outsb[:], 0.0)
            with tc.tile_pool(name="tmp2", bufs=4) as tmppool:
                for d, ps, zpool in agg_pass(tb2, 2):
                    z = zpool.tile([P, HID], F32, tag="z2")
                    nc.vector.tensor_scalar(out=z[:], in0=ps[:],
                                            scalar1=dinv_sb[:, d:d + 1],
                                            scalar2=None,
                                            op0=mybir.AluOpType.mult)
                    for ch in range(2):
                        tmp = tmppool.tile([P, HID], F32, tag="t2")
                        nc.vector.tensor_tensor(
                            out=tmp[:], in0=z[:],
                            in1=w2cb_sb[:, ch * HID:(ch + 1) * HID],
                            op=mybir.AluOpType.mult)
                        nc.vector.tensor_reduce(
                            out=outsb[:, 2 * d + ch:2 * d + ch + 1],
                            in_=tmp[:], axis=mybir.AxisListType.X,
                            op=mybir.AluOpType.add)

            nc.sync.dma_start(out=out_d[:, :], in_=outsb[:])

    nc.compile()
    return nc


def _prep(x, edge_index, W1, b1, W2):
    src = np.asarray(edge_index[0], dtype=np.int64)
    dst = np.asarray(edge_index[1], dtype=np.int64)
    loop = np.arange(N_NODES, dtype=np.int64)
    src = np.concatenate([src, loop])
    dst = np.concatenate([dst, loop])

    deg = np.bincount(dst, minlength=N_NODES)
    dinv = np.where(deg > 0, 1.0 / np.sqrt(deg.astype(np.float64)),
                    0.0).astype(np.float32)

    tiles_grd, key, counts, grow, dloc = _edge_structure(src, dst)

    t_call = tiles_grd.sum(axis=2)
    t_group = t_call.sum(axis=1)
    g_start = np.concatenate([[0], np.cumsum(t_group)])
    NT_TOT = int(g_start[-1])

    call_base = np.zeros((NG, NRANGE), np.int64)
    for g in range(NG):
        acc = int(g_start[g])
        for r in range(NRANGE):
            call_base[g, r] = acc
            acc += int(t_call[g, r])
    buck_base = np.zeros((NG, NRANGE, GB), np.int64)
    for g in range(NG):
        for r in range(NRANGE):
            acc = 0
            for dl in range(GB):
                buck_base[g, r, dl] = acc
                acc += int(tiles_grd[g, r, dl])

    order = np.argsort(key, kind="stable")
    starts = np.concatenate([[0], np.cumsum(counts.reshape(-1))])

    idx_all = np.zeros((CORES, P, 8 * NT_TOT), np.int16)
    dst_all = np.full((CORES, P, NT_TOT), -1.0, np.float32)
    kflat = 0
    for c in range(CORES):
        for g in range(NG):
            for r in range(NRANGE):
                for dl in range(GB):
                    s0, s1 = starts[kflat], starts[kflat + 1]
                    kflat += 1
                    cnt = s1 - s0
                    if cnt == 0:
                        continue
                    e = order[s0:s1]
                    J = buck_base[g, r, dl] * P + np.arange(cnt)
                    gt = call_base[g, r] + J // P
                    pp = J % P
                    dst_all[c, pp, gt] = dloc[e].astype(np.float32)
                    icol = 8 * call_base[g, r] + J // 16
                    idx_all[c, J % 16, icol] = (grow[e] - r * RANGE).astype(
                        np.int16)
    for q in range(1, 8):
        idx_all[:, 16 * q:16 * (q + 1), :] = idx_all[:, 0:16, :]

    dinv_pb = np.zeros((CORES, P, NB), np.float32)
    for c in range(CORES):
        n0, n1 = c * NSH, min((c + 1) * NSH, N_NODES)
        loc = np.zeros(NSH_PAD, np.float32)
        loc[: n1 - n0] = dinv[n0:n1]
        dinv_pb[c] = loc.reshape(NB, P).T

    xT = np.ascontiguousarray(np.asarray(x, np.float32).T.astype(ml_dtypes.bfloat16))
    b1t = np.tile(np.asarray(b1, np.float32)[None, :], (P, 1))
    w2 = np.asarray(W2, np.float32)
    w2cb = np.zeros((P, 2 * HID), np.float32)
    for ch in range(2):
        w2cb[:, ch * HID:(ch + 1) * HID] = np.tile(w2[:, ch][None, :], (P, 1))
    iota = np.tile(np.arange(P, dtype=np.float32)[None, :], (P, 1))
    W1f = np.asarray(W1, np.float32).astype(ml_dtypes.bfloat16)

    in_maps = []
    for c in range(CORES):
        n0, n1 = c * NSH, min((c + 1) * NSH, N_NODES)
        xtc = np.zeros((F_IN, NSH_PAD), ml_dtypes.bfloat16)
        xtc[:, : n1 - n0] = xT[:, n0:n1]
        in_maps.append({
            "xT": xtc, "W1": W1f, "B1T": b1t, "W2CB": w2cb, "IOTA": iota,
            "DINV": dinv_pb[c], "IDX": idx_all[c], "DSTLOC": dst_all[c],
        })
    return tiles_grd, in_maps


def kernel(x, edge_index, W1, b1, W2, b2):
    import os
    x = np.asarray(x)
    edge_index = np.asarray(edge_index)
    W1 = np.asarray(W1)
    b1 = np.asarray(b1)
    W2 = np.asarray(W2)
    b2 = np.asarray(b2, dtype=np.float32)
    assert x.shape == (N_NODES, F_IN), x.shape

    tiles_grd, in_maps = _prep(x, edge_index, W1, b1, W2)
    nc = _build_kernel(tiles_grd)
    trace = bool(int(os.environ.get("GCN_TRACE", "0")))
    try:
        res = run_bass_kernel_spmd(nc, in_maps, core_ids=list(range(CORES)),
                                   trace=trace)
    except Exception:
        if not trace:
            raise
        res = run_bass_kernel_spmd(nc, in_maps, core_ids=list(range(CORES)),
                                   trace=False)
    if trace and res.exec_time_ns is not None:
        print(f"HW exec time: {res.exec_time_ns} ns")

    out = np.zeros((N_NODES, 2), np.float32)
    for c in range(CORES):
        buf = res.results[c]["OUT"]
        arr = buf.reshape(P, NB, 2).transpose(1, 0, 2).reshape(NSH_PAD, 2)
        n0, n1 = c * NSH, min((c + 1) * NSH, N_NODES)
        out[n0:n1] = arr[: n1 - n0]
    return out + b2[None, :]

